# revision 10
# baseline (speedup 1.0000x reference)
"""Distributed embedding-lookup kernel (doc2vec PV-DM forward) for 8 trn2 cores.

Math (per batch element b):
    inputs[b,:]  = paragraph_matrix[doc_ids[b]] + mean_c word_matrix[context_ids[b,c]]
    result[b,s]  = dot(inputs[b,:], outputs[:, sample_ids[b,s]])

Sharding: data-parallel over batch (2048 rows/core).  Per the sharding hint
("doc ids are partitionable"), each core gets compact bf16 sub-tables holding
only the unique rows its batch slice touches:
  dcsub [18432,128] = [unique doc rows (cap 2048) | unique ctx word rows (cap
  16384)], ssub [20480,128] = unique sampled output columns (outputs is
  transposed host-side).  All sub-tables stay < 32768 rows so the gathers can
  use int16 indices.

Device gather: InstDMAGatherAnt (gpsimd.dma_gather, mlp ucode library) —
one instruction gathers thousands of rows (Q7 descriptor generation ~sub-ns
per row across 8 Q7 cores), vs. InstDMACopy-indirect's hard limit of 128
rows + ~1us fixed SWDGE cost per instruction (the old 466us baseline was
bound by that: 304 instructions/core).  single_packet=False is required for
>1024 idxs (single-packet mode overflows the 64-descriptor packet ceiling
and wedges the device).  bf16 tables halve HBM traffic and double DVE
throughput; overall rel-err ~3.5e-3 (gate 2e-2).

Per group of M=4 tiles (512 batch rows) the kernel issues 2 gathers:
  GA [128, 36, 128] <- dcsub  (chunks: doc t0..3 | ctx (c,t) c-major)
  GB [128, 40, 128] <- ssub   (chunks: smp (t,s) t-major)
then DVE: ctx tree-sum -> inputs = ctx/8 + doc -> broadcast-mult with the
sample block -> segmented f32 reduce -> one HWDGE write of [128, t, s].
"""

import sys

if "/opt/trn_rl_repo" not in sys.path:
    sys.path.insert(0, "/opt/trn_rl_repo")

import numpy as np

N_CORES = 8
B, C, S = 16384, 8, 10
D = 128
P = 128
N_DOCS, N_WORDS = 200000, 100000
BS = B // N_CORES   # 2048 batch rows per core
T = BS // P         # 16 tiles of 128 rows per core
M = 8               # tiles per group
G_CNT = T // M      # 4 groups
DC_CAP = BS         # doc-unique capacity
CW_CAP = BS * C     # ctx-word-unique capacity (16384)
DCSUB_ROWS = DC_CAP + CW_CAP   # 18432
SSUB_ROWS = BS * S             # 20480
NI_A = M * P * (1 + C)         # 4608 idxs per group (doc+ctx)
NI_B = M * P * S               # 5120 idxs per group (smp)

_COMPILED = {}
LAST_RESULT = None  # BassKernelResults of the most recent run (for test harness)


def build_program(reps=1):
    import concourse.bass as bass
    import concourse.tile as tile
    from concourse import bacc, mybir
    from contextlib import ExitStack

    f32 = mybir.dt.float32
    bf16 = mybir.dt.bfloat16
    i16 = mybir.dt.int16
    mult = mybir.AluOpType.mult
    add = mybir.AluOpType.add

    nc = bacc.Bacc(
        "TRN2",
        target_bir_lowering=False,
        debug=False,
        enable_asserts=False,
        num_devices=N_CORES,
        dynamic_dma_scratch_size=49152,
    )

    dcsub_d = nc.dram_tensor("dcsub", [DCSUB_ROWS, D], bf16, kind="ExternalInput").ap()
    ssub_d = nc.dram_tensor("ssub", [SSUB_ROWS, D], bf16, kind="ExternalInput").ap()
    idxa_d = nc.dram_tensor(
        "idxa", [P, G_CNT * (NI_A // 16)], i16, kind="ExternalInput"
    ).ap()
    idxb_d = nc.dram_tensor(
        "idxb", [P, G_CNT * (NI_B // 16)], i16, kind="ExternalInput"
    ).ap()
    res_d = nc.dram_tensor("res", [BS, S], f32, kind="ExternalOutput").ap()

    mD = M * D
    CA = NI_A // 16  # idxa cols per group (288)
    CB = NI_B // 16  # idxb cols per group (320)

    with tile.TileContext(nc) as tc, ExitStack() as ctx:
        idxp = ctx.enter_context(tc.tile_pool(name="idxp", bufs=1))
        gat = ctx.enter_context(tc.tile_pool(name="gat", bufs=2))
        cmp_p = ctx.enter_context(tc.tile_pool(name="cmp", bufs=1))
        outp = ctx.enter_context(tc.tile_pool(name="outp", bufs=2))

        idxa = idxp.tile([P, G_CNT * CA], i16, name="idxa")
        nc.sync.dma_start(out=idxa[:], in_=idxa_d)
        idxb = idxp.tile([P, G_CNT * CB], i16, name="idxb")
        nc.sync.dma_start(out=idxb[:], in_=idxb_d)

        def body():
            for g in range(G_CNT):
                GA = gat.tile([P, (1 + C) * mD], bf16, tag="GA", name="GA")
                nc.gpsimd.dma_gather(
                    out_ap=GA[:].rearrange("p (j e) -> p j e", j=(1 + C) * M, e=D),
                    in_ap=dcsub_d,
                    idxs_ap=idxa[:, g * CA : (g + 1) * CA],
                    num_idxs=NI_A,
                    num_idxs_reg=NI_A,
                    elem_size=D,
                    single_packet=False,
                )
                GB = gat.tile([P, S * mD], bf16, tag="GB", name="GB")
                nc.gpsimd.dma_gather(
                    out_ap=GB[:].rearrange("p (j e) -> p j e", j=S * M, e=D),
                    in_ap=ssub_d,
                    idxs_ap=idxb[:, g * CB : (g + 1) * CB],
                    num_idxs=NI_B,
                    num_idxs_reg=NI_B,
                    elem_size=D,
                    single_packet=False,
                )

                doc = GA[:, 0:mD]
                ctxb = GA[:, mD : (1 + C) * mD]

                # ctxsum = sum_c ctx_c  (tree over the c-major blocks)
                t1 = cmp_p.tile([P, 4 * mD], bf16, tag="t1", name="t1")
                nc.vector.tensor_add(
                    out=t1[:], in0=ctxb[:, 0 : 4 * mD], in1=ctxb[:, 4 * mD : 8 * mD]
                )
                t2 = cmp_p.tile([P, 2 * mD], bf16, tag="t2", name="t2")
                nc.vector.tensor_add(
                    out=t2[:], in0=t1[:, 0 : 2 * mD], in1=t1[:, 2 * mD : 4 * mD]
                )
                cs = cmp_p.tile([P, mD], bf16, tag="cs", name="cs")
                nc.vector.tensor_add(out=cs[:], in0=t2[:, 0:mD], in1=t2[:, mD : 2 * mD])

                # inp = ctxsum/C + doc
                inp = cmp_p.tile([P, mD], bf16, tag="inp", name="inp")
                nc.vector.scalar_tensor_tensor(
                    out=inp[:],
                    in0=cs[:],
                    scalar=1.0 / C,
                    in1=doc,
                    op0=mult,
                    op1=add,
                )

                # prod[p, t, s, :] = smp[p, t, s, :] * inp[p, t, :]
                prod = cmp_p.tile([P, S * mD], bf16, tag="prod", name="prod")
                smp4 = GB[:].rearrange("p (t s d) -> p t s d", t=M, s=S, d=D)
                inp4 = (
                    inp[:]
                    .rearrange("p (t d) -> p t d", t=M, d=D)
                    .unsqueeze(2)
                    .to_broadcast([P, M, S, D])
                )
                prod4 = prod[:].rearrange("p (t s d) -> p t s d", t=M, s=S, d=D)
                nc.vector.tensor_tensor(out=prod4, in0=smp4, in1=inp4, op=mult)

                # red[p, t*S+s] = sum_d prod[p, t, s, d]  (f32 accumulate)
                red = outp.tile([P, M * S], f32, tag="red", name="red")
                nc.vector.tensor_reduce(
                    out=red[:],
                    in_=prod[:].rearrange("p (ts d) -> p ts d", d=D),
                    axis=mybir.AxisListType.X,
                    op=add,
                )

                # res[(g*M+t)*P + p, s] = red[p, t*S+s]
                dst = res_d[g * M * P : (g + 1) * M * P, :].rearrange(
                    "(t p) s -> p t s", p=P
                )
                nc.sync.dma_start(out=dst, in_=red[:])

        if reps == 1:
            body()
        else:
            with tc.For_i(0, reps) as _i:
                body()

    nc.compile()
    return nc


def _get_program():
    if "nc" not in _COMPILED:
        _COMPILED["nc"] = build_program(1)
    return _COMPILED["nc"]


def _wrap16(pos_list):
    """[N] -> [128, N/16] int16: (ch, col) = pos[col*16+ch], replicated 8x
    (one copy per 16-partition group for the 8 Q7 descriptor-gen cores)."""
    w = np.asarray(pos_list, np.int16).reshape(-1, 16).T
    return np.tile(w, (8, 1))


def make_in_maps(doc_ids, context_ids, sample_ids, paragraph_matrix, word_matrix, outputs):
    import ml_dtypes

    bf = ml_dtypes.bfloat16
    par = np.asarray(paragraph_matrix, dtype=np.float32).astype(bf)
    wrd = np.asarray(word_matrix, dtype=np.float32).astype(bf)
    outT = np.ascontiguousarray(np.asarray(outputs, dtype=np.float32).T).astype(bf)
    doc_ids = np.asarray(doc_ids)
    context_ids = np.asarray(context_ids)
    sample_ids = np.asarray(sample_ids)

    in_maps = []
    for k in range(N_CORES):
        sl = slice(k * BS, (k + 1) * BS)
        du, dinv = np.unique(doc_ids[sl], return_inverse=True)
        cu, cinv = np.unique(context_ids[sl].ravel(), return_inverse=True)
        su, sinv = np.unique(sample_ids[sl].ravel(), return_inverse=True)
        assert len(du) <= DC_CAP and len(cu) <= CW_CAP and len(su) <= SSUB_ROWS

        dcsub = np.zeros((DCSUB_ROWS, D), bf)
        dcsub[: len(du)] = par[du]
        dcsub[DC_CAP : DC_CAP + len(cu)] = wrd[cu]
        ssub = np.zeros((SSUB_ROWS, D), bf)
        ssub[: len(su)] = outT[su]

        d = dinv.reshape(G_CNT, M, P)                      # [g, t, p]
        c = (cinv.reshape(G_CNT, M, P, C) + DC_CAP)        # [g, t, p, c]
        s = sinv.reshape(G_CNT, M, P, S)                   # [g, t, p, s]

        chunksA = np.concatenate(
            [d, c.transpose(0, 3, 1, 2).reshape(G_CNT, C * M, P)], axis=1
        )                                                  # [g, 36, p]
        chunksB = s.transpose(0, 1, 3, 2).reshape(G_CNT, S * M, P)  # [g, 40, p]

        idxa = np.concatenate(
            [_wrap16(chunksA[g].ravel()) for g in range(G_CNT)], axis=1
        )
        idxb = np.concatenate(
            [_wrap16(chunksB[g].ravel()) for g in range(G_CNT)], axis=1
        )
        in_maps.append(
            {
                "dcsub": dcsub,
                "ssub": ssub,
                "idxa": np.ascontiguousarray(idxa),
                "idxb": np.ascontiguousarray(idxb),
            }
        )
    return in_maps


def unshard_result(res_list):
    return np.concatenate(res_list, axis=0).astype(np.float32)


def kernel(
    doc_ids,
    context_ids,
    sample_ids,
    paragraph_matrix,
    word_matrix,
    outputs,
) -> np.ndarray:
    global LAST_RESULT
    from concourse.bass_utils import run_bass_kernel_spmd

    nc = _get_program()
    in_maps = make_in_maps(
        doc_ids, context_ids, sample_ids, paragraph_matrix, word_matrix, outputs
    )
    LAST_RESULT = run_bass_kernel_spmd(nc, in_maps, list(range(N_CORES)))
    return unshard_result(
        [LAST_RESULT.results[k]["res"] for k in range(N_CORES)]
    )


# revision 11
# speedup vs baseline: 1.1580x; 1.1580x over previous
"""Distributed embedding-lookup kernel (doc2vec PV-DM forward) for 8 trn2 cores.

Math (per batch element b):
    inputs[b,:]  = paragraph_matrix[doc_ids[b]] + mean_c word_matrix[context_ids[b,c]]
    result[b,s]  = dot(inputs[b,:], outputs[:, sample_ids[b,s]])

Sharding: data-parallel over batch (2048 rows/core).  Per the sharding hint
("doc ids are partitionable"), each core gets compact bf16 sub-tables holding
only the unique rows its batch slice touches:
  dcsub [18432,128] = [unique doc rows (cap 2048) | unique ctx word rows (cap
  16384)], ssub [20480,128] = unique sampled output columns (outputs is
  transposed host-side).  All sub-tables stay < 32768 rows so the gathers can
  use int16 indices.

Device gather: InstDMAGatherAnt (gpsimd.dma_gather, mlp ucode library) —
one instruction gathers thousands of rows (Q7 descriptor generation ~sub-ns
per row across 8 Q7 cores), vs. InstDMACopy-indirect's hard limit of 128
rows + ~1us fixed SWDGE cost per instruction (the old 466us baseline was
bound by that: 304 instructions/core).  single_packet=False is required for
>1024 idxs (single-packet mode overflows the 64-descriptor packet ceiling
and wedges the device).  bf16 tables halve HBM traffic and double DVE
throughput; overall rel-err ~3.5e-3 (gate 2e-2).

Per group of M=4 tiles (512 batch rows) the kernel issues 2 gathers:
  GA [128, 36, 128] <- dcsub  (chunks: doc t0..3 | ctx (c,t) c-major)
  GB [128, 40, 128] <- ssub   (chunks: smp (t,s) t-major)
then DVE: ctx tree-sum -> inputs = ctx/8 + doc -> broadcast-mult with the
sample block -> segmented f32 reduce -> one HWDGE write of [128, t, s].
"""

import sys

if "/opt/trn_rl_repo" not in sys.path:
    sys.path.insert(0, "/opt/trn_rl_repo")

import numpy as np

N_CORES = 8
B, C, S = 16384, 8, 10
D = 128
P = 128
N_DOCS, N_WORDS = 200000, 100000
BS = B // N_CORES   # 2048 batch rows per core
T = BS // P         # 16 tiles of 128 rows per core
M = 4               # tiles per group
G_CNT = T // M      # 4 groups
DC_CAP = BS         # doc-unique capacity
CW_CAP = BS * C     # ctx-word-unique capacity (16384)
DCSUB_ROWS = DC_CAP + CW_CAP   # 18432
SSUB_ROWS = BS * S             # 20480
NI_A = M * P * (1 + C)         # 4608 idxs per group (doc+ctx)
NI_B = M * P * S               # 5120 idxs per group (smp)

_COMPILED = {}
LAST_RESULT = None  # BassKernelResults of the most recent run (for test harness)


def build_program(reps=1):
    import concourse.bass as bass
    import concourse.tile as tile
    from concourse import bacc, mybir
    from contextlib import ExitStack

    f32 = mybir.dt.float32
    bf16 = mybir.dt.bfloat16
    i16 = mybir.dt.int16
    mult = mybir.AluOpType.mult
    add = mybir.AluOpType.add

    nc = bacc.Bacc(
        "TRN2",
        target_bir_lowering=False,
        debug=False,
        enable_asserts=False,
        num_devices=N_CORES,
        dynamic_dma_scratch_size=32768,
    )

    dcsub_d = nc.dram_tensor("dcsub", [DCSUB_ROWS, D], bf16, kind="ExternalInput").ap()
    ssub_d = nc.dram_tensor("ssub", [SSUB_ROWS, D], bf16, kind="ExternalInput").ap()
    idxa_d = nc.dram_tensor(
        "idxa", [P, G_CNT * (NI_A // 16)], i16, kind="ExternalInput"
    ).ap()
    idxb_d = nc.dram_tensor(
        "idxb", [P, G_CNT * (NI_B // 16)], i16, kind="ExternalInput"
    ).ap()
    res_d = nc.dram_tensor("res", [BS, S], f32, kind="ExternalOutput").ap()

    mD = M * D
    CA = NI_A // 16  # idxa cols per group (288)
    CB = NI_B // 16  # idxb cols per group (320)

    with tile.TileContext(nc) as tc, ExitStack() as ctx:
        idxp = ctx.enter_context(tc.tile_pool(name="idxp", bufs=1))
        gat = ctx.enter_context(tc.tile_pool(name="gat", bufs=3))
        cmp_p = ctx.enter_context(tc.tile_pool(name="cmp", bufs=2))
        outp = ctx.enter_context(tc.tile_pool(name="outp", bufs=2))

        idxa = idxp.tile([P, G_CNT * CA], i16, name="idxa")
        nc.sync.dma_start(out=idxa[:], in_=idxa_d)
        idxb = idxp.tile([P, G_CNT * CB], i16, name="idxb")
        nc.sync.dma_start(out=idxb[:], in_=idxb_d)

        def body():
            for g in range(G_CNT):
                GA = gat.tile([P, (1 + C) * mD], bf16, tag="GA", name="GA")
                nc.gpsimd.dma_gather(
                    out_ap=GA[:].rearrange("p (j e) -> p j e", j=(1 + C) * M, e=D),
                    in_ap=dcsub_d,
                    idxs_ap=idxa[:, g * CA : (g + 1) * CA],
                    num_idxs=NI_A,
                    num_idxs_reg=NI_A,
                    elem_size=D,
                    single_packet=False,
                )
                GB = gat.tile([P, S * mD], bf16, tag="GB", name="GB")
                nc.gpsimd.dma_gather(
                    out_ap=GB[:].rearrange("p (j e) -> p j e", j=S * M, e=D),
                    in_ap=ssub_d,
                    idxs_ap=idxb[:, g * CB : (g + 1) * CB],
                    num_idxs=NI_B,
                    num_idxs_reg=NI_B,
                    elem_size=D,
                    single_packet=False,
                )

                doc = GA[:, 0:mD]
                ctxb = GA[:, mD : (1 + C) * mD]

                # ctxsum = sum_c ctx_c  (tree over the c-major blocks)
                t1 = cmp_p.tile([P, 4 * mD], bf16, tag="t1", name="t1")
                nc.vector.tensor_add(
                    out=t1[:], in0=ctxb[:, 0 : 4 * mD], in1=ctxb[:, 4 * mD : 8 * mD]
                )
                t2 = cmp_p.tile([P, 2 * mD], bf16, tag="t2", name="t2")
                nc.vector.tensor_add(
                    out=t2[:], in0=t1[:, 0 : 2 * mD], in1=t1[:, 2 * mD : 4 * mD]
                )
                cs = cmp_p.tile([P, mD], bf16, tag="cs", name="cs")
                nc.vector.tensor_add(out=cs[:], in0=t2[:, 0:mD], in1=t2[:, mD : 2 * mD])

                # inp = ctxsum/C + doc
                inp = cmp_p.tile([P, mD], bf16, tag="inp", name="inp")
                nc.vector.scalar_tensor_tensor(
                    out=inp[:],
                    in0=cs[:],
                    scalar=1.0 / C,
                    in1=doc,
                    op0=mult,
                    op1=add,
                )

                # prod[p, t, s, :] = smp[p, t, s, :] * inp[p, t, :]
                prod = cmp_p.tile([P, S * mD], bf16, tag="prod", name="prod")
                smp4 = GB[:].rearrange("p (t s d) -> p t s d", t=M, s=S, d=D)
                inp4 = (
                    inp[:]
                    .rearrange("p (t d) -> p t d", t=M, d=D)
                    .unsqueeze(2)
                    .to_broadcast([P, M, S, D])
                )
                prod4 = prod[:].rearrange("p (t s d) -> p t s d", t=M, s=S, d=D)
                nc.vector.tensor_tensor(out=prod4, in0=smp4, in1=inp4, op=mult)

                # red[p, t*S+s] = sum_d prod[p, t, s, d]  (f32 accumulate)
                red = outp.tile([P, M * S], f32, tag="red", name="red")
                nc.vector.tensor_reduce(
                    out=red[:],
                    in_=prod[:].rearrange("p (ts d) -> p ts d", d=D),
                    axis=mybir.AxisListType.X,
                    op=add,
                )

                # res[(g*M+t)*P + p, s] = red[p, t*S+s]
                dst = res_d[g * M * P : (g + 1) * M * P, :].rearrange(
                    "(t p) s -> p t s", p=P
                )
                nc.sync.dma_start(out=dst, in_=red[:])

        if reps == 1:
            body()
        else:
            with tc.For_i(0, reps) as _i:
                body()

    nc.compile()
    return nc


def _get_program():
    if "nc" not in _COMPILED:
        _COMPILED["nc"] = build_program(1)
    return _COMPILED["nc"]


def _wrap16(pos_list):
    """[N] -> [128, N/16] int16: (ch, col) = pos[col*16+ch], replicated 8x
    (one copy per 16-partition group for the 8 Q7 descriptor-gen cores)."""
    w = np.asarray(pos_list, np.int16).reshape(-1, 16).T
    return np.tile(w, (8, 1))


def make_in_maps(doc_ids, context_ids, sample_ids, paragraph_matrix, word_matrix, outputs):
    import ml_dtypes

    bf = ml_dtypes.bfloat16
    par = np.asarray(paragraph_matrix, dtype=np.float32).astype(bf)
    wrd = np.asarray(word_matrix, dtype=np.float32).astype(bf)
    outT = np.ascontiguousarray(np.asarray(outputs, dtype=np.float32).T).astype(bf)
    doc_ids = np.asarray(doc_ids)
    context_ids = np.asarray(context_ids)
    sample_ids = np.asarray(sample_ids)

    in_maps = []
    for k in range(N_CORES):
        sl = slice(k * BS, (k + 1) * BS)
        du, dinv = np.unique(doc_ids[sl], return_inverse=True)
        cu, cinv = np.unique(context_ids[sl].ravel(), return_inverse=True)
        su, sinv = np.unique(sample_ids[sl].ravel(), return_inverse=True)
        assert len(du) <= DC_CAP and len(cu) <= CW_CAP and len(su) <= SSUB_ROWS

        dcsub = np.zeros((DCSUB_ROWS, D), bf)
        dcsub[: len(du)] = par[du]
        dcsub[DC_CAP : DC_CAP + len(cu)] = wrd[cu]
        ssub = np.zeros((SSUB_ROWS, D), bf)
        ssub[: len(su)] = outT[su]

        d = dinv.reshape(G_CNT, M, P)                      # [g, t, p]
        c = (cinv.reshape(G_CNT, M, P, C) + DC_CAP)        # [g, t, p, c]
        s = sinv.reshape(G_CNT, M, P, S)                   # [g, t, p, s]

        chunksA = np.concatenate(
            [d, c.transpose(0, 3, 1, 2).reshape(G_CNT, C * M, P)], axis=1
        )                                                  # [g, 36, p]
        chunksB = s.transpose(0, 1, 3, 2).reshape(G_CNT, S * M, P)  # [g, 40, p]

        idxa = np.concatenate(
            [_wrap16(chunksA[g].ravel()) for g in range(G_CNT)], axis=1
        )
        idxb = np.concatenate(
            [_wrap16(chunksB[g].ravel()) for g in range(G_CNT)], axis=1
        )
        in_maps.append(
            {
                "dcsub": dcsub,
                "ssub": ssub,
                "idxa": np.ascontiguousarray(idxa),
                "idxb": np.ascontiguousarray(idxb),
            }
        )
    return in_maps


def unshard_result(res_list):
    return np.concatenate(res_list, axis=0).astype(np.float32)


def kernel(
    doc_ids,
    context_ids,
    sample_ids,
    paragraph_matrix,
    word_matrix,
    outputs,
) -> np.ndarray:
    global LAST_RESULT
    from concourse.bass_utils import run_bass_kernel_spmd

    nc = _get_program()
    in_maps = make_in_maps(
        doc_ids, context_ids, sample_ids, paragraph_matrix, word_matrix, outputs
    )
    LAST_RESULT = run_bass_kernel_spmd(nc, in_maps, list(range(N_CORES)))
    return unshard_result(
        [LAST_RESULT.results[k]["res"] for k in range(N_CORES)]
    )


# revision 12
# speedup vs baseline: 1.1641x; 1.0052x over previous
"""Distributed embedding-lookup kernel (doc2vec PV-DM forward) for 8 trn2 cores.

Math (per batch element b):
    inputs[b,:]  = paragraph_matrix[doc_ids[b]] + mean_c word_matrix[context_ids[b,c]]
    result[b,s]  = dot(inputs[b,:], outputs[:, sample_ids[b,s]])

Sharding: data-parallel over batch (2048 rows/core).  Per the sharding hint
("doc ids are partitionable"), each core gets compact bf16 sub-tables holding
only the unique rows its batch slice touches:
  dcsub [18432,128] = [unique doc rows (cap 2048) | unique ctx word rows (cap
  16384)], ssub [20480,128] = unique sampled output columns (outputs is
  transposed host-side).  All sub-tables stay < 32768 rows so the gathers can
  use int16 indices.

Device gather: InstDMAGatherAnt (gpsimd.dma_gather, mlp ucode library) —
one instruction gathers thousands of rows (Q7 descriptor generation ~sub-ns
per row across 8 Q7 cores), vs. InstDMACopy-indirect's hard limit of 128
rows + ~1us fixed SWDGE cost per instruction (the old 466us baseline was
bound by that: 304 instructions/core).  single_packet=False is required for
>1024 idxs (single-packet mode overflows the 64-descriptor packet ceiling
and wedges the device).  bf16 tables halve HBM traffic and double DVE
throughput; overall rel-err ~3.5e-3 (gate 2e-2).

Per group of M=4 tiles (512 batch rows) the kernel issues 2 gathers:
  GA [128, 36, 128] <- dcsub  (chunks: doc t0..3 | ctx (c,t) c-major)
  GB [128, 40, 128] <- ssub   (chunks: smp (t,s) t-major)
then DVE: ctx tree-sum -> inputs = ctx/8 + doc -> broadcast-mult with the
sample block -> segmented f32 reduce -> one HWDGE write of [128, t, s].
"""

import sys

if "/opt/trn_rl_repo" not in sys.path:
    sys.path.insert(0, "/opt/trn_rl_repo")

import numpy as np

N_CORES = 8
B, C, S = 16384, 8, 10
D = 128
P = 128
N_DOCS, N_WORDS = 200000, 100000
BS = B // N_CORES   # 2048 batch rows per core
T = BS // P         # 16 tiles of 128 rows per core
M = 4               # tiles per group
G_CNT = T // M      # 4 groups
DC_CAP = BS         # doc-unique capacity
CW_CAP = BS * C     # ctx-word-unique capacity (16384)
DCSUB_ROWS = DC_CAP + CW_CAP   # 18432
SSUB_ROWS = BS * S             # 20480
NI_A = M * P * (1 + C)         # 4608 idxs per group (doc+ctx)
NI_B = M * P * S               # 5120 idxs per group (smp)

_COMPILED = {}
LAST_RESULT = None  # BassKernelResults of the most recent run (for test harness)


def build_program(reps=1):
    import concourse.bass as bass
    import concourse.tile as tile
    from concourse import bacc, mybir
    from contextlib import ExitStack

    f32 = mybir.dt.float32
    bf16 = mybir.dt.bfloat16
    i16 = mybir.dt.int16
    mult = mybir.AluOpType.mult
    add = mybir.AluOpType.add

    nc = bacc.Bacc(
        "TRN2",
        target_bir_lowering=False,
        debug=False,
        enable_asserts=False,
        num_devices=N_CORES,
    )

    dcsub_d = nc.dram_tensor("dcsub", [DCSUB_ROWS, D], bf16, kind="ExternalInput").ap()
    ssub_d = nc.dram_tensor("ssub", [SSUB_ROWS, D], bf16, kind="ExternalInput").ap()
    idxa_d = nc.dram_tensor(
        "idxa", [P, G_CNT * (NI_A // 16)], i16, kind="ExternalInput"
    ).ap()
    idxb_d = nc.dram_tensor(
        "idxb", [P, G_CNT * (NI_B // 16)], i16, kind="ExternalInput"
    ).ap()
    res_d = nc.dram_tensor("res", [BS, S], f32, kind="ExternalOutput").ap()

    mD = M * D
    CA = NI_A // 16  # idxa cols per group (288)
    CB = NI_B // 16  # idxb cols per group (320)

    with tile.TileContext(nc) as tc, ExitStack() as ctx:
        idxp = ctx.enter_context(tc.tile_pool(name="idxp", bufs=1))
        gat = ctx.enter_context(tc.tile_pool(name="gat", bufs=3))
        cmp_p = ctx.enter_context(tc.tile_pool(name="cmp", bufs=2))
        outp = ctx.enter_context(tc.tile_pool(name="outp", bufs=2))

        idxa = idxp.tile([P, G_CNT * CA], i16, name="idxa")
        nc.sync.dma_start(out=idxa[:], in_=idxa_d)
        idxb = idxp.tile([P, G_CNT * CB], i16, name="idxb")
        nc.sync.dma_start(out=idxb[:], in_=idxb_d)

        def body():
            for g in range(G_CNT):
                GA = gat.tile([P, (1 + C) * mD], bf16, tag="GA", name="GA")
                nc.gpsimd.dma_gather(
                    out_ap=GA[:].rearrange("p (j e) -> p j e", j=(1 + C) * M, e=D),
                    in_ap=dcsub_d,
                    idxs_ap=idxa[:, g * CA : (g + 1) * CA],
                    num_idxs=NI_A,
                    num_idxs_reg=NI_A,
                    elem_size=D,
                    single_packet=False,
                )
                GB = gat.tile([P, S * mD], bf16, tag="GB", name="GB")
                nc.gpsimd.dma_gather(
                    out_ap=GB[:].rearrange("p (j e) -> p j e", j=S * M, e=D),
                    in_ap=ssub_d,
                    idxs_ap=idxb[:, g * CB : (g + 1) * CB],
                    num_idxs=NI_B,
                    num_idxs_reg=NI_B,
                    elem_size=D,
                    single_packet=False,
                )

                doc = GA[:, 0:mD]
                ctxb = GA[:, mD : (1 + C) * mD]

                # ctxsum = sum_c ctx_c  (tree over the c-major blocks)
                t1 = cmp_p.tile([P, 4 * mD], bf16, tag="t1", name="t1")
                nc.vector.tensor_add(
                    out=t1[:], in0=ctxb[:, 0 : 4 * mD], in1=ctxb[:, 4 * mD : 8 * mD]
                )
                t2 = cmp_p.tile([P, 2 * mD], bf16, tag="t2", name="t2")
                nc.vector.tensor_add(
                    out=t2[:], in0=t1[:, 0 : 2 * mD], in1=t1[:, 2 * mD : 4 * mD]
                )
                cs = cmp_p.tile([P, mD], bf16, tag="cs", name="cs")
                nc.vector.tensor_add(out=cs[:], in0=t2[:, 0:mD], in1=t2[:, mD : 2 * mD])

                # inp = ctxsum/C + doc
                inp = cmp_p.tile([P, mD], bf16, tag="inp", name="inp")
                nc.vector.scalar_tensor_tensor(
                    out=inp[:],
                    in0=cs[:],
                    scalar=1.0 / C,
                    in1=doc,
                    op0=mult,
                    op1=add,
                )

                # prod[p, t, s, :] = smp[p, t, s, :] * inp[p, t, :]
                prod = cmp_p.tile([P, S * mD], bf16, tag="prod", name="prod")
                smp4 = GB[:].rearrange("p (t s d) -> p t s d", t=M, s=S, d=D)
                inp4 = (
                    inp[:]
                    .rearrange("p (t d) -> p t d", t=M, d=D)
                    .unsqueeze(2)
                    .to_broadcast([P, M, S, D])
                )
                prod4 = prod[:].rearrange("p (t s d) -> p t s d", t=M, s=S, d=D)
                nc.vector.tensor_tensor(out=prod4, in0=smp4, in1=inp4, op=mult)

                # red[p, t*S+s] = sum_d prod[p, t, s, d]  (f32 accumulate)
                red = outp.tile([P, M * S], f32, tag="red", name="red")
                nc.vector.tensor_reduce(
                    out=red[:],
                    in_=prod[:].rearrange("p (ts d) -> p ts d", d=D),
                    axis=mybir.AxisListType.X,
                    op=add,
                )

                # res[(g*M+t)*P + p, s] = red[p, t*S+s]
                dst = res_d[g * M * P : (g + 1) * M * P, :].rearrange(
                    "(t p) s -> p t s", p=P
                )
                nc.sync.dma_start(out=dst, in_=red[:])

        if reps == 1:
            body()
        else:
            with tc.For_i(0, reps) as _i:
                body()

    nc.compile()
    return nc


def _get_program():
    if "nc" not in _COMPILED:
        _COMPILED["nc"] = build_program(1)
    return _COMPILED["nc"]


def _wrap16(pos_list):
    """[N] -> [128, N/16] int16: (ch, col) = pos[col*16+ch], replicated 8x
    (one copy per 16-partition group for the 8 Q7 descriptor-gen cores)."""
    w = np.asarray(pos_list, np.int16).reshape(-1, 16).T
    return np.tile(w, (8, 1))


def make_in_maps(doc_ids, context_ids, sample_ids, paragraph_matrix, word_matrix, outputs):
    import ml_dtypes

    bf = ml_dtypes.bfloat16
    par = np.asarray(paragraph_matrix, dtype=np.float32).astype(bf)
    wrd = np.asarray(word_matrix, dtype=np.float32).astype(bf)
    outT = np.ascontiguousarray(np.asarray(outputs, dtype=np.float32).T).astype(bf)
    doc_ids = np.asarray(doc_ids)
    context_ids = np.asarray(context_ids)
    sample_ids = np.asarray(sample_ids)

    in_maps = []
    for k in range(N_CORES):
        sl = slice(k * BS, (k + 1) * BS)
        du, dinv = np.unique(doc_ids[sl], return_inverse=True)
        cu, cinv = np.unique(context_ids[sl].ravel(), return_inverse=True)
        su, sinv = np.unique(sample_ids[sl].ravel(), return_inverse=True)
        assert len(du) <= DC_CAP and len(cu) <= CW_CAP and len(su) <= SSUB_ROWS

        dcsub = np.zeros((DCSUB_ROWS, D), bf)
        dcsub[: len(du)] = par[du]
        dcsub[DC_CAP : DC_CAP + len(cu)] = wrd[cu]
        ssub = np.zeros((SSUB_ROWS, D), bf)
        ssub[: len(su)] = outT[su]

        d = dinv.reshape(G_CNT, M, P)                      # [g, t, p]
        c = (cinv.reshape(G_CNT, M, P, C) + DC_CAP)        # [g, t, p, c]
        s = sinv.reshape(G_CNT, M, P, S)                   # [g, t, p, s]

        chunksA = np.concatenate(
            [d, c.transpose(0, 3, 1, 2).reshape(G_CNT, C * M, P)], axis=1
        )                                                  # [g, 36, p]
        chunksB = s.transpose(0, 1, 3, 2).reshape(G_CNT, S * M, P)  # [g, 40, p]

        idxa = np.concatenate(
            [_wrap16(chunksA[g].ravel()) for g in range(G_CNT)], axis=1
        )
        idxb = np.concatenate(
            [_wrap16(chunksB[g].ravel()) for g in range(G_CNT)], axis=1
        )
        in_maps.append(
            {
                "dcsub": dcsub,
                "ssub": ssub,
                "idxa": np.ascontiguousarray(idxa),
                "idxb": np.ascontiguousarray(idxb),
            }
        )
    return in_maps


def unshard_result(res_list):
    return np.concatenate(res_list, axis=0).astype(np.float32)


def kernel(
    doc_ids,
    context_ids,
    sample_ids,
    paragraph_matrix,
    word_matrix,
    outputs,
) -> np.ndarray:
    global LAST_RESULT
    from concourse.bass_utils import run_bass_kernel_spmd

    nc = _get_program()
    in_maps = make_in_maps(
        doc_ids, context_ids, sample_ids, paragraph_matrix, word_matrix, outputs
    )
    LAST_RESULT = run_bass_kernel_spmd(nc, in_maps, list(range(N_CORES)))
    return unshard_result(
        [LAST_RESULT.results[k]["res"] for k in range(N_CORES)]
    )


# revision 13
# speedup vs baseline: 3.7071x; 3.1846x over previous
"""Distributed embedding-lookup kernel (doc2vec PV-DM forward) for 8 trn2 cores.

Math (per batch element b):
    inputs[b,:]  = paragraph_matrix[doc_ids[b]] + mean_c word_matrix[context_ids[b,c]]
    result[b,s]  = dot(inputs[b,:], outputs[:, sample_ids[b,s]])

Sharding: data-parallel over batch (2048 rows/core).  Per the sharding hint
("doc ids are partitionable"), each core gets compact bf16 sub-tables holding
only the unique rows its batch slice touches:
  dcsub [18432,128] = [unique doc rows (cap 2048) | unique ctx word rows (cap
  16384)], ssub [20480,128] = unique sampled output columns (outputs is
  transposed host-side).  All sub-tables stay < 32768 rows so the gathers can
  use int16 indices.

Device gather: InstDMAGatherAnt (gpsimd.dma_gather, mlp ucode library) —
one instruction gathers thousands of rows (Q7 descriptor generation ~sub-ns
per row across 8 Q7 cores), vs. InstDMACopy-indirect's hard limit of 128
rows + ~1us fixed SWDGE cost per instruction (the old 466us baseline was
bound by that: 304 instructions/core).  single_packet=False is required for
>1024 idxs (single-packet mode overflows the 64-descriptor packet ceiling
and wedges the device).  bf16 tables halve HBM traffic and double DVE
throughput; overall rel-err ~3.5e-3 (gate 2e-2).

Per group of M=4 tiles (512 batch rows) the kernel issues 2 gathers:
  GA [128, 36, 128] <- dcsub  (chunks: doc t0..3 | ctx (c,t) c-major)
  GB [128, 40, 128] <- ssub   (chunks: smp (t,s) t-major)
then DVE: ctx tree-sum -> inputs = ctx/8 + doc -> broadcast-mult with the
sample block -> segmented f32 reduce -> one HWDGE write of [128, t, s].
"""

import sys

if "/opt/trn_rl_repo" not in sys.path:
    sys.path.insert(0, "/opt/trn_rl_repo")

import numpy as np

N_CORES = 8
B, C, S = 16384, 8, 10
D = 128
P = 128
N_DOCS, N_WORDS = 200000, 100000
BS = B // N_CORES   # 2048 batch rows per core
T = BS // P         # 16 tiles of 128 rows per core
M = 4               # tiles per group
G_CNT = T // M      # 4 groups
DC_CAP = BS         # doc-unique capacity
CW_CAP = BS * C     # ctx-word-unique capacity (16384)
DCSUB_ROWS = DC_CAP + CW_CAP   # 18432
SSUB_ROWS = BS * S             # 20480
NI_A = M * P * (1 + C)         # 4608 idxs per group (doc+ctx)
NI_B = M * P * S               # 5120 idxs per group (smp)

_COMPILED = {}
LAST_RESULT = None  # BassKernelResults of the most recent run (for test harness)


def build_program(reps=1):
    import concourse.bass as bass
    import concourse.tile as tile
    from concourse import bacc, mybir
    from contextlib import ExitStack

    f32 = mybir.dt.float32
    bf16 = mybir.dt.bfloat16
    i16 = mybir.dt.int16
    mult = mybir.AluOpType.mult
    add = mybir.AluOpType.add

    nc = bacc.Bacc(
        "TRN2",
        target_bir_lowering=False,
        debug=False,
        enable_asserts=False,
        num_devices=N_CORES,
        num_swdge_queues=4,
    )

    dcsub_d = nc.dram_tensor("dcsub", [DCSUB_ROWS, D], bf16, kind="ExternalInput").ap()
    ssub_d = nc.dram_tensor("ssub", [SSUB_ROWS, D], bf16, kind="ExternalInput").ap()
    idxa_d = nc.dram_tensor(
        "idxa", [P, G_CNT * (NI_A // 16)], i16, kind="ExternalInput"
    ).ap()
    idxb_d = nc.dram_tensor(
        "idxb", [P, G_CNT * (NI_B // 16)], i16, kind="ExternalInput"
    ).ap()
    res_d = nc.dram_tensor("res", [BS, S], f32, kind="ExternalOutput").ap()

    mD = M * D
    CA = NI_A // 16  # idxa cols per group (288)
    CB = NI_B // 16  # idxb cols per group (320)

    with tile.TileContext(nc) as tc, ExitStack() as ctx:
        idxp = ctx.enter_context(tc.tile_pool(name="idxp", bufs=1))
        gat = ctx.enter_context(tc.tile_pool(name="gat", bufs=3))
        cmp_p = ctx.enter_context(tc.tile_pool(name="cmp", bufs=2))
        outp = ctx.enter_context(tc.tile_pool(name="outp", bufs=2))

        idxa = idxp.tile([P, G_CNT * CA], i16, name="idxa")
        nc.sync.dma_start(out=idxa[:], in_=idxa_d)
        idxb = idxp.tile([P, G_CNT * CB], i16, name="idxb")
        nc.sync.dma_start(out=idxb[:], in_=idxb_d)

        def body():
            for g in range(G_CNT):
                # Each gather is split across the 4 SWDGE queues: the SDMA
                # engines interleave packets from different queues, so random
                # 256B row reads get ~4 outstanding HBM requests per engine
                # instead of 1 (measured 3.5x gather throughput).
                GA = gat.tile([P, (1 + C) * mD], bf16, tag="GA", name="GA")
                na = NI_A // 4
                for q in range(4):
                    nc.gpsimd.dma_gather(
                        out_ap=GA[
                            :, q * (na // 128) * D : (q + 1) * (na // 128) * D
                        ].rearrange("p (j e) -> p j e", j=na // 128, e=D),
                        in_ap=dcsub_d,
                        idxs_ap=idxa[
                            :, g * CA + q * (na // 16) : g * CA + (q + 1) * (na // 16)
                        ],
                        num_idxs=na,
                        num_idxs_reg=na,
                        elem_size=D,
                        single_packet=False,
                        queue_num=q,
                    )
                GB = gat.tile([P, S * mD], bf16, tag="GB", name="GB")
                nb = NI_B // 4
                for q in range(4):
                    nc.gpsimd.dma_gather(
                        out_ap=GB[
                            :, q * (nb // 128) * D : (q + 1) * (nb // 128) * D
                        ].rearrange("p (j e) -> p j e", j=nb // 128, e=D),
                        in_ap=ssub_d,
                        idxs_ap=idxb[
                            :, g * CB + q * (nb // 16) : g * CB + (q + 1) * (nb // 16)
                        ],
                        num_idxs=nb,
                        num_idxs_reg=nb,
                        elem_size=D,
                        single_packet=False,
                        queue_num=q,
                    )

                doc = GA[:, 0:mD]
                ctxb = GA[:, mD : (1 + C) * mD]

                # ctxsum = sum_c ctx_c  (tree over the c-major blocks)
                t1 = cmp_p.tile([P, 4 * mD], bf16, tag="t1", name="t1")
                nc.vector.tensor_add(
                    out=t1[:], in0=ctxb[:, 0 : 4 * mD], in1=ctxb[:, 4 * mD : 8 * mD]
                )
                t2 = cmp_p.tile([P, 2 * mD], bf16, tag="t2", name="t2")
                nc.vector.tensor_add(
                    out=t2[:], in0=t1[:, 0 : 2 * mD], in1=t1[:, 2 * mD : 4 * mD]
                )
                cs = cmp_p.tile([P, mD], bf16, tag="cs", name="cs")
                nc.vector.tensor_add(out=cs[:], in0=t2[:, 0:mD], in1=t2[:, mD : 2 * mD])

                # inp = ctxsum/C + doc
                inp = cmp_p.tile([P, mD], bf16, tag="inp", name="inp")
                nc.vector.scalar_tensor_tensor(
                    out=inp[:],
                    in0=cs[:],
                    scalar=1.0 / C,
                    in1=doc,
                    op0=mult,
                    op1=add,
                )

                # prod[p, t, s, :] = smp[p, t, s, :] * inp[p, t, :]
                prod = cmp_p.tile([P, S * mD], bf16, tag="prod", name="prod")
                smp4 = GB[:].rearrange("p (t s d) -> p t s d", t=M, s=S, d=D)
                inp4 = (
                    inp[:]
                    .rearrange("p (t d) -> p t d", t=M, d=D)
                    .unsqueeze(2)
                    .to_broadcast([P, M, S, D])
                )
                prod4 = prod[:].rearrange("p (t s d) -> p t s d", t=M, s=S, d=D)
                nc.vector.tensor_tensor(out=prod4, in0=smp4, in1=inp4, op=mult)

                # red[p, t*S+s] = sum_d prod[p, t, s, d]  (f32 accumulate)
                red = outp.tile([P, M * S], f32, tag="red", name="red")
                nc.vector.tensor_reduce(
                    out=red[:],
                    in_=prod[:].rearrange("p (ts d) -> p ts d", d=D),
                    axis=mybir.AxisListType.X,
                    op=add,
                )

                # res[(g*M+t)*P + p, s] = red[p, t*S+s]
                dst = res_d[g * M * P : (g + 1) * M * P, :].rearrange(
                    "(t p) s -> p t s", p=P
                )
                nc.sync.dma_start(out=dst, in_=red[:])

        if reps == 1:
            body()
        else:
            with tc.For_i(0, reps) as _i:
                body()

    nc.compile()
    return nc


def _get_program():
    if "nc" not in _COMPILED:
        _COMPILED["nc"] = build_program(1)
    return _COMPILED["nc"]


def _wrap16(pos_list):
    """[N] -> [128, N/16] int16: (ch, col) = pos[col*16+ch], replicated 8x
    (one copy per 16-partition group for the 8 Q7 descriptor-gen cores)."""
    w = np.asarray(pos_list, np.int16).reshape(-1, 16).T
    return np.tile(w, (8, 1))


def make_in_maps(doc_ids, context_ids, sample_ids, paragraph_matrix, word_matrix, outputs):
    import ml_dtypes

    bf = ml_dtypes.bfloat16
    par = np.asarray(paragraph_matrix, dtype=np.float32).astype(bf)
    wrd = np.asarray(word_matrix, dtype=np.float32).astype(bf)
    outT = np.ascontiguousarray(np.asarray(outputs, dtype=np.float32).T).astype(bf)
    doc_ids = np.asarray(doc_ids)
    context_ids = np.asarray(context_ids)
    sample_ids = np.asarray(sample_ids)

    in_maps = []
    for k in range(N_CORES):
        sl = slice(k * BS, (k + 1) * BS)
        du, dinv = np.unique(doc_ids[sl], return_inverse=True)
        cu, cinv = np.unique(context_ids[sl].ravel(), return_inverse=True)
        su, sinv = np.unique(sample_ids[sl].ravel(), return_inverse=True)
        assert len(du) <= DC_CAP and len(cu) <= CW_CAP and len(su) <= SSUB_ROWS

        dcsub = np.zeros((DCSUB_ROWS, D), bf)
        dcsub[: len(du)] = par[du]
        dcsub[DC_CAP : DC_CAP + len(cu)] = wrd[cu]
        ssub = np.zeros((SSUB_ROWS, D), bf)
        ssub[: len(su)] = outT[su]

        d = dinv.reshape(G_CNT, M, P)                      # [g, t, p]
        c = (cinv.reshape(G_CNT, M, P, C) + DC_CAP)        # [g, t, p, c]
        s = sinv.reshape(G_CNT, M, P, S)                   # [g, t, p, s]

        chunksA = np.concatenate(
            [d, c.transpose(0, 3, 1, 2).reshape(G_CNT, C * M, P)], axis=1
        )                                                  # [g, 36, p]
        chunksB = s.transpose(0, 1, 3, 2).reshape(G_CNT, S * M, P)  # [g, 40, p]

        idxa = np.concatenate(
            [_wrap16(chunksA[g].ravel()) for g in range(G_CNT)], axis=1
        )
        idxb = np.concatenate(
            [_wrap16(chunksB[g].ravel()) for g in range(G_CNT)], axis=1
        )
        in_maps.append(
            {
                "dcsub": dcsub,
                "ssub": ssub,
                "idxa": np.ascontiguousarray(idxa),
                "idxb": np.ascontiguousarray(idxb),
            }
        )
    return in_maps


def unshard_result(res_list):
    return np.concatenate(res_list, axis=0).astype(np.float32)


def kernel(
    doc_ids,
    context_ids,
    sample_ids,
    paragraph_matrix,
    word_matrix,
    outputs,
) -> np.ndarray:
    global LAST_RESULT
    from concourse.bass_utils import run_bass_kernel_spmd

    nc = _get_program()
    in_maps = make_in_maps(
        doc_ids, context_ids, sample_ids, paragraph_matrix, word_matrix, outputs
    )
    LAST_RESULT = run_bass_kernel_spmd(nc, in_maps, list(range(N_CORES)))
    return unshard_result(
        [LAST_RESULT.results[k]["res"] for k in range(N_CORES)]
    )
